# revision 1
# baseline (speedup 1.0000x reference)
"""Multi-head attention Bass/Tile kernel for Trainium2 (8 NeuronCores, SPMD).

Reference semantics (note the reference's intentional name swap):
    k = split_heads(query @ Wk.T); q = split_heads(key @ Wq.T)
    v = split_heads(value @ Wv.T)
    wei = (q @ k^T) * C**-0.5
    wei = where(mask == 0, 0.0, wei)        # masked scores become 0, NOT -inf
    wei = softmax(wei, axis=-1)             # masked entries contribute exp(0)=1
    out = (wei @ v)  -> merge heads -> @ Wproj.T + bproj

Strategy: data-parallel over batch B=8 (one batch element per core).
Per core, scores are computed transposed (S^T[k, q]) per head so that the
softmax numerator matmul (P^T stationary against V) needs no on-chip
transposes of the big P matrix.  The pre-softmax masking-with-zero is
handled algebraically:
    p = exp(m * s) = m * (exp(s) - 1) + 1      (m in {0,1})
so for fully-unmasked 128x128 blocks p = exp(s) directly; for fully-masked
blocks p = 1 (skip everything, add colsum(V) via an all-ones stationary
matmul); for mixed blocks one fused DVE op (affine_mul_reduce) computes
p~ = (exp(s) - 1) * m and the same all-ones correction matmul adds the +1
part.  The block classification is done on the host from the actual mask
input, so the compiled program is specialized to the mask pattern (cached).
"""

import os
import sys

sys.path.insert(0, "/opt/trn_rl_repo")

import numpy as np

B, T, C = 8, 2048, 384
H, D = 6, 64
VW = D + 1  # per-head V width incl. the ones column (softmax denominator)
SCALE = float(C) ** -0.5
P = 128  # partitions / block edge
QCH = 512  # q chunk for the S matmul (moving operand, fp32r needs >=256)

ZERO, ONES, MIXED = 0, 1, 2

_CACHE = {}
LAST_PROFILE = {}


def _build_program(t, cls, mixed_ids, n_mixed, repeat=1, phases="all", qk_dtype="f32r"):
    """Build the SPMD Bass program for sequence length t (t % QCH == 0).

    cls: [t/128][t/128] block classification (kb-major: cls[kb][qt]).
    mixed_ids: dict (kb, qt) -> index into the packed mixed-mask input.
    repeat: if > 1, wrap the whole kernel in a hardware loop (timing only).
    """
    from contextlib import ExitStack

    import concourse.mybir as mybir
    import concourse.tile as tile
    from concourse import bacc
    from concourse.masks import make_identity

    f32 = mybir.dt.float32
    f32r = mybir.dt.float32r
    bf16 = mybir.dt.bfloat16

    nt = t // P  # number of 128-blocks along tokens
    qch = min(QCH, t)
    nqc = t // qch

    nc = bacc.Bacc()

    xqT = nc.dram_tensor("xqT", [C, t], f32r, kind="ExternalInput")
    xkT = nc.dram_tensor("xkT", [C, t], f32r, kind="ExternalInput")
    xvT = nc.dram_tensor("xvT", [C, t], f32r, kind="ExternalInput")
    wkT = nc.dram_tensor("wkT", [C, C], f32r, kind="ExternalInput")
    wqT = nc.dram_tensor("wqT", [C, C], f32r, kind="ExternalInput")
    wvT = nc.dram_tensor("wvT", [C, C], f32r, kind="ExternalInput")
    wpT = nc.dram_tensor("wpT", [C, C], f32r, kind="ExternalInput")
    bpj = nc.dram_tensor("bpj", [1, C], f32, kind="ExternalInput")
    if n_mixed:
        mmT = nc.dram_tensor("mmT", [n_mixed, P, P], bf16, kind="ExternalInput")

    # per-(kb) live q-range (multiples of 128), and per-qt live kb list
    qlive = []  # (qlo, qhi) in token units, or None
    for kb in range(nt):
        lv = [qt for qt in range(nt) if cls[kb][qt] != ZERO]
        qlive.append((lv[0] * P, (lv[-1] + 1) * P) if lv else None)
    corr_kbs = [[kb for kb in range(nt) if cls[kb][qt] != ONES] for qt in range(nt)]
    live_kbs = [[kb for kb in range(nt) if cls[kb][qt] != ZERO] for qt in range(nt)]
    y = nc.dram_tensor("y", [t, C], f32, kind="ExternalOutput")

    with ExitStack() as ctx:
        tc = ctx.enter_context(tile.TileContext(nc))
        if repeat > 1:
            ctx.enter_context(tc.For_i(0, repeat, 1))
        consts = ctx.enter_context(tc.tile_pool(name="consts", bufs=1))

        # ---- persistent constants -------------------------------------
        w_s = {}
        tl = consts.tile([P, 3, C], f32r, tag="wp", name="w_wp")
        for c in range(3):
            nc.sync.dma_start(out=tl[:, c, :], in_=wpT[c * P : (c + 1) * P, :])
        w_s["wp"] = tl
        bias_s = consts.tile([P, C], f32, tag="bias")
        nc.sync.dma_start(out=bias_s, in_=bpj[:, :].to_broadcast((P, C)))
        ones_col = consts.tile([P, 1], bf16, tag="ones_col")
        nc.vector.memset(ones_col, 1.0)
        ones_row = consts.tile([1, P], f32, tag="ones_row")
        nc.vector.memset(ones_row, 1.0)
        ident = consts.tile([P, P], bf16, tag="ident")
        make_identity(nc, ident)
        any_corr = any(corr_kbs[qt] for qt in range(nt))
        if n_mixed:
            mm_s = consts.tile([P, n_mixed, P], bf16, tag="mm")
            nc.sync.dma_start(
                out=mm_s, in_=mmT[:, :, :].rearrange("n p f -> p n f")
            )

        # persistent activations: Q^T, K^T (feature-major), V' (token-major)
        qk_dt = f32r if qk_dtype == "f32r" else bf16
        qT_s = consts.tile([P, 3, t], qk_dt, tag="qT")
        kT_s = consts.tile([P, 3, t], qk_dt, tag="kT")
        vp_s = consts.tile([P, nt, H * VW], bf16, tag="vp")

        c_sb = None
        # ---- phase A: projections -------------------------------------
        # Order matters: the PE executes in program order, so emit the work
        # whose results unblock ScalarE (the bottleneck engine) last: V'
        # projection + corrections first, then Q^T/K^T, then attention.
        with tc.tile_pool(name="xt", bufs=2) as xt_pool, tc.tile_pool(
            name="wqkv", bufs=1
        ) as wqkv_pool, tc.tile_pool(name="pproj", bufs=2, space="PSUM") as pproj:
            for name, dram in (("wv", wvT), ("wk", wkT), ("wq", wqT)):
                tl = wqkv_pool.tile([P, 3, C], f32r, tag=name, name=f"w_{name}")
                for c in range(3):
                    nc.sync.dma_start(
                        out=tl[:, c, :], in_=dram[c * P : (c + 1) * P, :]
                    )
                w_s[name] = tl
            # V (token-major) = xvT.T @ wvT, written per head with a ones col
            xs = []
            for c in range(3):
                xt_tile = xt_pool.tile([P, t], f32r, tag=f"x{c}", name=f"xv{c}")
                nc.sync.dma_start(out=xt_tile, in_=xvT[c * P : (c + 1) * P, :])
                xs.append(xt_tile)
            for tt in range(nt):
                ps = pproj.tile([P, C], f32, tag="ppv")
                for c in range(3):
                    nc.tensor.matmul(
                        ps,
                        lhsT=xs[c][:, tt * P : (tt + 1) * P],
                        rhs=w_s["wv"][:, c, :],
                        start=(c == 0),
                        stop=(c == 2),
                    )
                # strided copy: psum [128, (h d)] -> vp[:, tt, h*VW : h*VW+D]
                nc.any.tensor_copy(
                    out=vp_s[:, tt, :].rearrange("p (h w) -> p h w", h=H)[
                        :, :, 0:D
                    ],
                    in_=ps.rearrange("p (h d) -> p h d", h=H),
                )
            nc.vector.memset(
                vp_s.rearrange("p n (h w) -> p n h w", h=H)[:, :, :, D : D + 1], 1.0
            )

            # per-qt correction vectors (overlap with Q^T/K^T projections)
            # C[qt] = sum of colsum(V' block kb) over non-ONES blocks kb.
            # Process qts smallest-set-first; when the next set is a superset
            # of the previous one (always true for causal masks), keep
            # accumulating into the same PSUM tile and only add the diff.
            if any_corr:
                with tc.tile_pool(name="cpsum", bufs=2, space="PSUM") as cpool:
                    c_sb = consts.tile([1, nt, H * VW], f32, tag="c_sb")
                    order = sorted(
                        (qt for qt in range(nt) if corr_kbs[qt]),
                        key=lambda q: len(corr_kbs[q]),
                    )
                    prev = None
                    c_ps = None
                    for qt in order:
                        s = set(corr_kbs[qt])
                        if prev is not None and prev <= s:
                            add = sorted(s - prev)
                            fresh = False
                        else:
                            add = sorted(s)
                            fresh = True
                            c_ps = cpool.tile([1, H * VW], f32, tag="cps",
                                              name=f"cps{qt}")
                        for i, kb in enumerate(add):
                            nc.tensor.matmul(
                                c_ps,
                                lhsT=ones_col,
                                rhs=vp_s[:, kb, :],
                                start=(fresh and i == 0),
                                stop=(i == len(add) - 1),
                                skip_group_check=True,
                            )
                        nc.vector.tensor_copy(out=c_sb[:, qt, :], in_=c_ps)
                        prev = s

            # Q^T = wqT.T @ xkT   (reference swap: q comes from the `key` input)
            # K^T = wkT.T @ xqT
            for wname, xdram, dst in (("wq", xkT, qT_s), ("wk", xqT, kT_s)):
                xs = []
                for c in range(3):
                    xt_tile = xt_pool.tile([P, t], f32r, tag=f"x{c}", name=f"x{c}")
                    nc.sync.dma_start(
                        out=xt_tile, in_=xdram[c * P : (c + 1) * P, :]
                    )
                    xs.append(xt_tile)
                for i in range(3):  # output feature chunk
                    for j in range(nqc):  # token chunk
                        ps = pproj.tile([P, QCH], f32, tag="pp")
                        for c in range(3):  # contraction chunk
                            nc.tensor.matmul(
                                ps,
                                lhsT=w_s[wname][:, c, i * P : (i + 1) * P],
                                rhs=xs[c][:, j * QCH : (j + 1) * QCH],
                                start=(c == 0),
                                stop=(c == 2),
                            )
                        nc.any.tensor_copy(
                            out=dst[:, i, j * QCH : (j + 1) * QCH], in_=ps
                        )

        # ---- phase B/C: attention -------------------------------------
        obig = {}
        with tc.tile_pool(name="obig", bufs=nt) as obig_pool:
            for qt in range(nt):
                obig[qt] = obig_pool.tile([P, C], bf16, tag="ob", name=f"obig{qt}")

            # size-classed e-tile pools: tiles sized to each key-block's live
            # q-range so more than one head's worth fits in SBUF (overlap
            # exp of head h+1 with the PV matmuls of head h)
            ecls = {}
            counts = {}
            for kb in range(nt):
                if qlive[kb] is None:
                    continue
                w = qlive[kb][1] - qlive[kb][0]
                wc = -(-w // QCH) * QCH
                ecls[kb] = wc
                counts[wc] = counts.get(wc, 0) + 1
            base = sum(wc * n for wc, n in counts.items()) * P * 2  # bytes
            budget = 11 * 2 ** 20
            f = min(2.0, budget / max(base, 1))
            ebufs = {
                wc: n + (max(1, int(n * (f - 1.0))) if f > 1.02 else 0)
                for wc, n in counts.items()
            }

            SW = 1024  # S-psum window width (2 banks each, 3 bufs)
            with ExitStack() as pools:
                spool = pools.enter_context(
                    tc.tile_pool(name="spsum", bufs=3, space="PSUM"))
                opool = pools.enter_context(
                    tc.tile_pool(name="opsum", bufs=2, space="PSUM"))
                e_pools = {
                    wc: pools.enter_context(
                        tc.tile_pool(name=f"e{wc}", bufs=ebufs[wc]))
                    for wc in counts
                }
                pmix = pools.enter_context(
                    tc.tile_pool(name="pmix", bufs=max(4, min(n_mixed, nt * H)) + 2))
                scr_pool = pools.enter_context(tc.tile_pool(name="scr", bufs=4))
                norm_pool = pools.enter_context(tc.tile_pool(name="norm", bufs=4))

                for h in range(H):
                    hp = (h * D) // P  # which 128-chunk of features
                    ho = (h * D) % P  # offset inside it (0 or 64)
                    e_tiles = {}
                    p_tiles = {}
                    for kb in range(nt):
                        if qlive[kb] is None:
                            continue
                        qlo, qhi = qlive[kb]
                        wc = ecls[kb]
                        et = e_pools[wc].tile([P, wc], bf16, tag=f"e{wc}",
                                              name=f"e_{h}_{kb}")
                        for w0 in range(0, t, SW):
                            w1 = min(w0 + SW, t)
                            if w1 <= qlo or w0 >= qhi:
                                continue
                            sp = spool.tile([P, SW], f32, tag="s",
                                            name=f"s_{h}_{kb}_{w0}")
                            for j in range(nqc):
                                j0, j1 = j * qch, (j + 1) * qch
                                if j1 <= max(qlo, w0) or j0 >= min(qhi, w1):
                                    continue
                                nc.tensor.matmul(
                                    sp[:, j0 - w0 : j1 - w0],
                                    lhsT=kT_s[
                                        ho : ho + D, hp, kb * P : (kb + 1) * P
                                    ],
                                    rhs=qT_s[ho : ho + D, hp, j0:j1],
                                    start=True,
                                    stop=True,
                                )
                            r0, r1 = max(qlo, w0), min(qhi, w1)
                            nc.scalar.activation(
                                out=et[:, r0 - qlo : r1 - qlo],
                                in_=sp[:, r0 - w0 : r1 - w0],
                                func=mybir.ActivationFunctionType.Exp,
                                scale=SCALE,
                            )
                        e_tiles[kb] = (et, qlo)
                        for qt in range(nt):
                            if cls[kb][qt] != MIXED:
                                continue
                            pt = pmix.tile([P, P], bf16, tag="pm")
                            sc = scr_pool.tile([P, 1], f32, tag="sc")
                            nc.vector.affine_mul_reduce(
                                out=pt,
                                accum_out=sc,
                                in0=et[:, qt * P - qlo : (qt + 1) * P - qlo],
                                in1=mm_s[:, mixed_ids[(kb, qt)], :],
                                scale=1.0,
                                bias=-1.0,
                            )
                            p_tiles[(kb, qt)] = pt

                    for qt in range(nt if phases == "all" else 0):
                        op = opool.tile([P, VW], f32, tag="op")
                        has_c = bool(corr_kbs[qt])
                        nmm = (1 if has_c else 0) + len(live_kbs[qt])
                        i = 0
                        if has_c:
                            # broadcast-add C[qt] to every q row (rank-1 matmul)
                            nc.tensor.matmul(
                                op,
                                lhsT=ones_row,
                                rhs=c_sb[:, qt, h * VW : (h + 1) * VW],
                                start=True,
                                stop=(nmm == 1),
                            )
                            i += 1
                        for kb in live_kbs[qt]:
                            if cls[kb][qt] == MIXED:
                                lhsT = p_tiles[(kb, qt)][:]
                            else:
                                et, qlo = e_tiles[kb]
                                lhsT = et[:, qt * P - qlo : (qt + 1) * P - qlo]
                            nc.tensor.matmul(
                                op,
                                lhsT=lhsT,
                                rhs=vp_s[:, kb, h * VW : (h + 1) * VW],
                                start=(i == 0),
                                stop=(i == nmm - 1),
                            )
                            i += 1
                        rc = norm_pool.tile([P, 1], f32, tag="rc")
                        nc.vector.reciprocal(rc, op[:, D : D + 1])
                        nc.vector.tensor_scalar_mul(
                            out=obig[qt][:, h * D : (h + 1) * D],
                            in0=op[:, 0:D],
                            scalar1=rc[:],
                        )

            # ---- phase D: output projection (own PSUM scope: the banks
            # are recycled from the attention pools closed above) --------
            with ExitStack() as dpools:
                if phases != "all":
                    ysb0 = dpools.enter_context(tc.tile_pool(name="ysb0", bufs=1))
                    zt = ysb0.tile([P, C], f32, tag="z")
                    nc.vector.memset(zt, 0.0)
                    for qt in range(nt):
                        nc.sync.dma_start(out=y[qt * P : (qt + 1) * P, :], in_=zt)
                otpool = dpools.enter_context(
                    tc.tile_pool(name="otp", bufs=2, space="PSUM"))
                ypool = dpools.enter_context(
                    tc.tile_pool(name="yp", bufs=2, space="PSUM"))
                ots_pool = dpools.enter_context(tc.tile_pool(name="otsp", bufs=3))
                ysb_pool = dpools.enter_context(tc.tile_pool(name="ysb", bufs=3))
                for qt in range(nt if phases == "all" else 0):
                    otp = otpool.tile([P, C], bf16, tag="ot")
                    for c in range(3):
                        nc.tensor.transpose(
                            out=otp[:, c * P : (c + 1) * P],
                            in_=obig[qt][:, c * P : (c + 1) * P],
                            identity=ident,
                        )
                    ots = ots_pool.tile([P, C], f32r, tag="ots")
                    nc.any.tensor_copy(out=ots, in_=otp)
                    yps = ypool.tile([P, C], f32, tag="y")
                    for c in range(3):
                        nc.tensor.matmul(
                            yps,
                            lhsT=ots[:, c * P : (c + 1) * P],
                            rhs=w_s["wp"][:, c, :],
                            start=(c == 0),
                            stop=(c == 2),
                        )
                    ysb = ysb_pool.tile([P, C], f32, tag="ysb")
                    nc.any.tensor_add(out=ysb, in0=yps, in1=bias_s)
                    nc.sync.dma_start(out=y[qt * P : (qt + 1) * P, :], in_=ysb)

    nc.finalize()
    return nc


def _classify_mask(mask2d, t):
    """Host-side classification of the [t, t] 0/1 mask into 128x128 blocks.

    Returns cls[kb][qt], mixed block index map, and the packed transposed
    bf16 mixed blocks ([n, 128, 128], m^T layout: [k, q])."""
    import ml_dtypes

    nt = t // P
    m = mask2d.reshape(nt, P, nt, P)  # [qt, qp, kb, kp]
    any_ = m.any(axis=(1, 3))  # [qt, kb]
    all_ = m.all(axis=(1, 3))
    cls = [[ZERO] * nt for _ in range(nt)]
    mixed_ids = {}
    blocks = []
    for kb in range(nt):
        for qt in range(nt):
            if all_[qt, kb]:
                cls[kb][qt] = ONES
            elif any_[qt, kb]:
                cls[kb][qt] = MIXED
                mixed_ids[(kb, qt)] = len(blocks)
                blocks.append(
                    np.ascontiguousarray(m[qt, :, kb, :].T).astype(
                        ml_dtypes.bfloat16
                    )
                )
    packed = np.stack(blocks) if blocks else None
    return cls, mixed_ids, packed


def kernel(query, key, value, mask, Wk, Wq, Wv, Wproj, bproj):
    from concourse.bass_utils import run_bass_kernel_spmd

    query = np.asarray(query, dtype=np.float32)
    key = np.asarray(key, dtype=np.float32)
    value = np.asarray(value, dtype=np.float32)
    b, t, c = query.shape
    mask2d = np.asarray(mask, dtype=np.int32).reshape(t, t) != 0

    cls, mixed_ids, packed = _classify_mask(mask2d, t)
    n_mixed = 0 if packed is None else len(packed)

    cache_key = (t, bytes(bytearray(v for row in cls for v in row)))
    if cache_key not in _CACHE:
        _CACHE[cache_key] = _build_program(t, cls, mixed_ids, n_mixed)
    nc = _CACHE[cache_key]

    wk = np.ascontiguousarray(np.asarray(Wk, np.float32).T)
    wq = np.ascontiguousarray(np.asarray(Wq, np.float32).T)
    wv = np.ascontiguousarray(np.asarray(Wv, np.float32).T)
    wp = np.ascontiguousarray(np.asarray(Wproj, np.float32).T)
    bp = np.asarray(bproj, np.float32).reshape(1, c)

    in_maps = []
    for i in range(b):
        m = {
            "xqT": np.ascontiguousarray(query[i].T),
            "xkT": np.ascontiguousarray(key[i].T),
            "xvT": np.ascontiguousarray(value[i].T),
            "wkT": wk,
            "wqT": wq,
            "wvT": wv,
            "wpT": wp,
            "bpj": bp,
        }
        if n_mixed:
            m["mmT"] = packed
        in_maps.append(m)

    trace = bool(int(os.environ.get("BASS_MHA_TRACE", "0")))
    res = run_bass_kernel_spmd(nc, in_maps, core_ids=list(range(b)), trace=trace)
    LAST_PROFILE.clear()
    LAST_PROFILE.update(
        exec_time_ns=res.exec_time_ns,
        mean_exec_time_ns=res.mean_exec_time_ns,
        trace=res.instructions_and_trace,
    )
    return np.stack([res.results[i]["y"] for i in range(b)])

